# revision 37
# baseline (speedup 1.0000x reference)
"""Trainium2 Bass kernel for nn_ExpertGQALayer (dense transformer layer:
RMSNorm -> GQA attention with RoPE -> residual -> RMSNorm -> SwiGLU MLP -> residual).

Sharding: pure data-parallel over batch. B=8 batch elements, 8 NeuronCores,
one batch element per core. No collectives.

Device-side dataflow keeps every activation in transposed [feature, token]
layout so that all matmul contractions sit on the SBUF partition axis.

Key speed levers vs the bf16 baseline (HW-measured: fresh-stationary bf16
MM = ~291ns, fp8 DoubleRow pair-MM = ~323ns == 1.80x per unit work):
  * q/k/v/o projections run in fp8e4 with perf_mode=DoubleRow, contracting
    two 128-chunks per instruction (weights pre-interleaved host-side,
    scaled by 64 to clear the e4m3 subnormal region).
  * All partition-axis reductions/broadcasts (rmsnorm mean, softmax
    denominator) moved off the PE onto GPSIMD partition_all_reduce, and all
    norm/softmax broadcast matmuls eliminated (the all-reduce output is
    already broadcast across partitions).
  * A global x64 scale rides the residual stream (xt64 = 64*x) so every
    fp8-weight PSUM result needs no extra descale op: the 1/64 folds into
    the RoPE cos table, the V/Q/K evacuation scales, the rsqrt chain
    (rinv/64), and the down-projection weights (wd*64); the host divides
    the output by 64.
  * norm2 square-sums accumulate eagerly inside the o-projection loop so
    the MLP starts ~immediately after the attention residual completes.

Attention scores/PV and the whole MLP stay bf16 (fp8 there fails the 2e-2
tolerance; measured on CPU: mlp-fp8 => 4.3e-2, attention-fp8 => 5e-3).
"""

import math
from contextlib import ExitStack

import ml_dtypes
import numpy as np

import concourse.mybir as mybir
import concourse.tile as tile
from concourse import bacc, bass_isa
from concourse.bass_utils import run_bass_kernel_spmd

# Problem dimensions (hardcoded per contest contract)
B, S, H = 8, 512, 2048
NQ, NKV, HD, INTER = 16, 4, 128, 8192
GROUPS = NQ // NKV
MAX_SEQ = 512
THETA = 100000.0
EPS = 1e-6
SCALE = 1.0 / math.sqrt(HD)

P = 128
KT = H // P          # 16 contraction tiles over H
KP = KT // 2         # 8 DoubleRow pair-tiles over H
IT = INTER // P      # 64 contraction tiles over INTER
TCH = S // P         # 4 token chunks

SW = 64.0            # fp8 weight scale (and the residual-stream scale)

f32 = mybir.dt.float32
bf16 = mybir.dt.bfloat16
fp8 = mybir.dt.float8e4
bf16_np = ml_dtypes.bfloat16
fp8_np = ml_dtypes.float8_e4m3

AF = mybir.ActivationFunctionType
DR = mybir.MatmulPerfMode.DoubleRow
RADD = bass_isa.ReduceOp.add


def _emit(tc, t, first=True):
    """Emit the per-core program. t: dict of DRAM APs."""
    nc = tc.nc

    with ExitStack() as octx:
        # ---- pools that live for the whole kernel ----
        glob = octx.enter_context(tc.tile_pool(name="glob", bufs=1))
        bca = octx.enter_context(tc.tile_pool(name="bca", bufs=1))

        cosT = glob.tile([P, S], f32)   # 64*cos (x rinv1 = cos*r on-device)
        sinT = glob.tile([P, S], f32)    # 64*sin
        perm = glob.tile([P, P], bf16)
        x2T = glob.tile([P, KT, S], bf16)  # 64*(attention-block residual stream)
        one11 = nc.const_aps.tensor(1.0, (1, 1), f32)
        ident = glob.tile([P, P], bf16)  # identity for PE transposes (phase 2)
        nc.sync.dma_start(ident[:], t["ident"])
        eps_t = glob.tile([P, 1], f32)
        nc.any.memset(eps_t[:], EPS * SW * SW)  # eps for 64^2-scaled square sums

        def sq_accum(src_k, k, accs):
            """Two parallel square-sum chains (even k: ACT square + DVE add,
            odd k: DVE square + DVE add) so the per-chunk dependency chain
            never exceeds the chunk-arrival cadence. accs: [even, odd]."""
            sq = sqp.tile([P, S], f32, tag="sq")
            if k % 2 == 0:
                nc.scalar.activation(sq[:], src_k, AF.Square)
            else:
                nc.vector.tensor_mul(sq[:], src_k, src_k)
            eng = nc.vector
            if accs[k % 2] is None:
                accs[k % 2] = sq
            else:
                acc = accp.tile([P, S], f32, tag="acc")
                eng.tensor_add(acc[:], accs[k % 2][:], sq[:])
                accs[k % 2] = acc
            return accs

        def combine_accs(accs):
            acc = accp.tile([P, S], f32, tag="acc")
            nc.vector.tensor_add(acc[:], accs[0][:], accs[1][:])
            return acc

        def rinv64_from_acc(acc, sum_scale):
            """acc: [P,S] f32 partial square-sums (per-partition).
            Returns [P,S] f32 tile = rsqrt(mean_h(x^2)+eps)/64, broadcast on
            all partitions (GPSIMD all-reduce handles both the cross-partition
            sum and the broadcast). sum_scale converts the all-reduced sum to
            4096*mean (4096/H if acc holds true-scale squares, 1/H if the
            squares came from the 64x-scaled stream)."""
            ssum = bca.tile([P, S], f32, tag="ssum")
            nc.gpsimd.partition_all_reduce(ssum[:], acc[:], channels=P, reduce_op=RADD)
            srow = bca.tile([P, S], f32, tag="srow")
            # = 64*sqrt(mean+eps)
            nc.scalar.activation(srow[:], ssum[:], AF.Sqrt, bias=eps_t[:], scale=sum_scale)
            rinv = bca.tile([P, S], f32, tag="rinv")
            nc.vector.reciprocal_approx_fast(rinv[:], srow[:])
            return rinv

        # ================= phase 1: attention =================
        with ExitStack() as ctx:
            ph1 = ctx.enter_context(tc.tile_pool(name="ph1", bufs=1))
            sqp = ctx.enter_context(tc.tile_pool(name="sqp", bufs=3))
            accp = ctx.enter_context(tc.tile_pool(name="accp", bufs=3))
            wst = ctx.enter_context(tc.tile_pool(name="wst", bufs=3))
            psA = ctx.enter_context(tc.tile_pool(name="psA", bufs=4, space="PSUM"))
            ropep = ctx.enter_context(tc.tile_pool(name="ropep", bufs=2))
            ep = ctx.enter_context(tc.tile_pool(name="ep", bufs=7))
            esp = ctx.enter_context(tc.tile_pool(name="esp", bufs=3))

            # x8 first: the unnormalized projections consume it immediately.
            # The f32 stream (residual) is only needed by the o-projection.
            # activations/tables ride the Activation-engine DMA issue stream so
            # the SP stream is free to dispatch weight tiles immediately
            x8 = ph1.tile([P, KT, S], fp8)  # fp8(x^T), true scale
            if first:
                for k in range(0, KT, 4):
                    nc.scalar.dma_start(x8[:, k : k + 4], t["x8"][:, k : k + 4])
            nc.sync.dma_start(cosT[:], t["cosT"])
            nc.sync.dma_start(sinT[:], t["sinT"])
            nc.sync.dma_start(perm[:], t["perm"])
            if first:
                xt = ph1.tile([P, KT, S], bf16)  # bf16(64*x^T); only o-proj needs it
                for k in range(0, KT, 8):
                    nc.sync.dma_start(xt[:, k : k + 8], t["xt"][:, k : k + 8])
            else:
                # depth-chained (timing-only) layers read the f32 mid tensor and
                # derive the fp8 stream on-device
                xt = ph1.tile([P, KT, S], f32)
                for k in range(0, KT, 8):
                    nc.sync.dma_start(xt[:, k : k + 8], t["xt"][:, k : k + 8])
                for k in range(KT):
                    eng = nc.scalar if k % 2 == 0 else nc.vector
                    if k % 2 == 0:
                        nc.scalar.activation(x8[:, k], xt[:, k], AF.Copy, scale=1.0 / SW)
                    else:
                        nc.vector.tensor_scalar_mul(x8[:, k], xt[:, k], 1.0 / SW)

            # norm1 square-sums from the (early, small) fp8 stream; the fp8
            # quantization adds ~0.1% noise to r, negligible.
            accs1 = [None, None]
            for k in range(KT):
                sq_accum(x8[:, k], k, accs1)
            rinv1 = rinv64_from_acc(combine_accs(accs1), SW * SW / H)  # rsqrt/64

            # RoPE tables with the rmsnorm scale folded in (r commutes with
            # the H-contraction): projections run on raw x8, and the per-token
            # scale r rides in via cos/sin (q/k) and the V-evacuation scale.
            cos_r = glob.tile([P, S], bf16)  # = cos * r  (cosT = 64*cos host-side)
            nc.vector.tensor_mul(cos_r[:], cosT[:], rinv1[:])
            sin_r = glob.tile([P, S], bf16)  # = sin * r   (sinT = 64*sin host-side)
            nc.vector.tensor_mul(sin_r[:], sinT[:], rinv1[:])

            q_all = ph1.tile([P, NQ, S], bf16)
            k_all = ph1.tile([P, NKV, S], bf16)
            v_all = ph1.tile([P, TCH, NKV * HD], bf16)
            o8 = ph1.tile([P, NQ, S], fp8)

            with tc.tile_pool(name="psB", bufs=2, space="PSUM") as psB:
                def project_and_rope(w_dram, n_heads, dst):
                    # head-pairs: one weight tile (4 queue-split DMAs on an
                    # alternating issue engine) feeds two heads' matmuls
                    for hp in range(n_heads // 2):
                        wt = wst.tile([P, 2, KP, 2, HD], fp8, tag="w")
                        nc.sync.dma_start(wt[:], w_dram[hp])
                        for hh in range(2):
                            psq = psA.tile([P, S], f32, tag="acc")  # = 64*q_unnorm
                            for kp in range(KP):
                                nc.tensor.matmul(
                                    psq[:], wt[:, hh, kp], x8[:, 2 * kp : 2 * kp + 2, :],
                                    start=(kp == 0), stop=(kp == KP - 1), perf_mode=DR,
                                )
                            # RoPE+norm: dst = (psq/64)*r*cos + (perm@(psq/64))*r*sin
                            qs = ropep.tile([P, S], bf16, tag="qs")
                            nc.scalar.activation(qs[:], psq[:], AF.Copy, scale=1.0 / SW)
                            psr = psB.tile([P, S], f32, tag="bc")
                            nc.tensor.matmul(psr[:], perm[:], qs[:], start=True, stop=True)
                            t1 = ropep.tile([P, S], bf16, tag="t1")
                            nc.vector.tensor_mul(t1[:], qs[:], cos_r[:])
                            t2 = ropep.tile([P, S], bf16, tag="t2")
                            nc.vector.tensor_mul(t2[:], psr[:], sin_r[:])
                            nc.vector.tensor_add(dst[:, 2 * hp + hh], t1[:], t2[:])

                project_and_rope(t["wq_t"], NQ, q_all)
                project_and_rope(t["wk_t"], NKV, k_all)

                # per-token-chunk columns of r/64 for the V evacuation scale
                rcol = glob.tile([P, TCH], f32)  # rcol[:, tc] = r/64 for chunk tc
                for tc_ in range(TCH):
                    psT = psB.tile([P, 1], f32, tag="bc")
                    nc.tensor.transpose(
                        psT[:], rinv1[0:1, tc_ * P : (tc_ + 1) * P], one11
                    )
                    nc.vector.tensor_copy(rcol[:, tc_ : tc_ + 1], psT[:])

                wv_sb = ph1.tile([P, KP, 2, NKV * HD], fp8)
                nc.sync.dma_start(wv_sb[:, :4], t["wv_t"][:, :4])
                nc.sync.dma_start(wv_sb[:, 4:], t["wv_t"][:, 4:])

                for tc_ in range(TCH):
                    psv = psA.tile([P, NKV * HD], f32, tag="acc")  # 64*v_unnorm [t,d]
                    for kp in range(KP):
                        nc.tensor.matmul(
                            psv[:],
                            x8[:, 2 * kp : 2 * kp + 2, tc_ * P : (tc_ + 1) * P],
                            wv_sb[:, kp],
                            start=(kp == 0), stop=(kp == KP - 1), perf_mode=DR,
                        )
                    # v = (64*v_unnorm) * (r/64) per token-partition
                    nc.scalar.activation(
                        v_all[:, tc_], psv[:], AF.Copy, scale=rcol[:, tc_ : tc_ + 1]
                    )

            # attention per kv-group (scores get their own PSUM pool so head
            # n+1's score matmuls never wait on head n's softmax tail)
            with tc.tile_pool(name="psS", bufs=4, space="PSUM") as psS:
              for g in range(NKV):
                for h in range(g * GROUPS, (g + 1) * GROUPS):
                    e_list = []
                    for tc_ in range(TCH):
                        pss = psS.tile([P, S], f32, tag="sc")
                        nc.tensor.matmul(
                            pss[:],
                            k_all[:, g, tc_ * P : (tc_ + 1) * P],
                            q_all[:, h],
                            start=True, stop=True,
                        )
                        e = ep.tile([P, S], bf16, tag="e")
                        nc.scalar.activation(e[:], pss[:], AF.Exp, scale=SCALE)
                        e_list.append(e)
                    # PV first: keeps PE busy while DVE/GPSIMD do the denominator
                    pso = psA.tile([P, S], f32, tag="acc")
                    for tc_ in range(TCH):
                        nc.tensor.matmul(
                            pso[:],
                            v_all[:, tc_, g * HD : (g + 1) * HD],
                            e_list[tc_][:],
                            start=(tc_ == 0), stop=(tc_ == TCH - 1),
                        )
                    # denominator: DVE tree over the 4 chunks, then GPSIMD
                    # all-reduce across partitions (output already broadcast)
                    s01 = esp.tile([P, S], bf16, tag="esum")
                    nc.vector.tensor_add(s01[:], e_list[0][:], e_list[1][:])
                    s23 = esp.tile([P, S], bf16, tag="esum")
                    nc.vector.tensor_add(s23[:], e_list[2][:], e_list[3][:])
                    s03 = esp.tile([P, S], bf16, tag="esum")
                    nc.vector.tensor_add(s03[:], s01[:], s23[:])
                    den = bca.tile([P, S], f32, tag="ssum")
                    nc.gpsimd.partition_all_reduce(den[:], s03[:], channels=P, reduce_op=RADD)
                    rec = bca.tile([P, S], f32, tag="rec")
                    nc.vector.reciprocal_approx_fast(rec[:], den[:])
                    nc.vector.tensor_mul(o8[:, h], pso[:], rec[:])

            if "dbg_h1" in t:
                nc.sync.dma_start(t["dbg_h1"], x8[:])
                nc.sync.dma_start(t["dbg_q"], q_all[:])
                nc.sync.dma_start(t["dbg_k"], k_all[:])
                nc.sync.dma_start(t["dbg_v"], v_all[:])
                nc.sync.dma_start(t["dbg_o8"], o8[:])
            # o-projection + residual -> x2T (= 64*x2); eager norm2 square-sums
            accs2 = [None, None]
            for mp in range(KT // 2):
                wt = wst.tile([P, 2, KP, 2, P], fp8, tag="w")
                nc.sync.dma_start(wt[:], t["wo_t"][mp])
                for hh in range(2):
                    m = 2 * mp + hh
                    pso = psA.tile([P, S], f32, tag="acc")  # = 64*(wo@o)
                    for jp in range(KP):
                        nc.tensor.matmul(
                            pso[:], wt[:, hh, jp], o8[:, 2 * jp : 2 * jp + 2, :],
                            start=(jp == 0), stop=(jp == KP - 1), perf_mode=DR,
                        )
                    nc.vector.tensor_add(x2T[:, m], pso[:], xt[:, m])
                    sq_accum(x2T[:, m], m, accs2)

            # rinv2 = r2/64 (bca tile, outlives this scope); the gate/up
            # matmuls consume x2T with r2 applied at evacuation time.
            rinv2 = rinv64_from_acc(combine_accs(accs2), 1.0 / H)

        if "dbg_x2" in t:
            nc.sync.dma_start(t["dbg_x2"], x2T[:])
        # ================= phase 2: MLP =================
        with ExitStack() as ctx:
            ph2 = ctx.enter_context(tc.tile_pool(name="ph2", bufs=1))
            wgup = ctx.enter_context(tc.tile_pool(name="wgup", bufs=10))
            wdp = ctx.enter_context(tc.tile_pool(name="wdp", bufs=1))
            sgp = ctx.enter_context(tc.tile_pool(name="sgp", bufs=2))
            otp = ctx.enter_context(tc.tile_pool(name="otp", bufs=1))

            # gate/up run act-stationary on the 64x-scaled x2T (one LDWEIGHTS
            # per 4 matmuls: j-pair x {gate,up} share each stationary
            # token-chunk), producing token-major g/u in PSUM; the per-token
            # r2 rides in via the silu/copy evacuation scale, and PE
            # transposes put `a` back in feature-major for the down-proj.
            a_all = ph2.tile([P, IT, S], bf16)

            with tc.tile_pool(name="psG", bufs=8, space="PSUM") as psG:
                rcol2 = glob.tile([P, TCH], f32)  # = r2/64 per token column
                for tc_ in range(TCH):
                    psT = psG.tile([P, 1], f32, tag="acc", name="psTrc")
                    nc.tensor.transpose(
                        psT[:], rinv2[0:1, tc_ * P : (tc_ + 1) * P], one11
                    )
                    nc.vector.tensor_copy(rcol2[:, tc_ : tc_ + 1], psT[:])

                for jp in range(8):  # j-pairs; j = 512 gate + 512 up features
                    wjs = [[None] * 4, [None] * 4]
                    for q in range(4):  # K-quarter streaming
                        for jj in range(2):
                            w = wgup.tile([P, KT // 4, 2, 512], bf16, tag="wj")
                            nc.sync.dma_start(w[:], t["wgu_t"][2 * jp + jj, q])
                            wjs[jj][q] = w
                    for sc in range(TCH):
                        # one token-chunk per burst: its evacuation chain
                        # (silu/copy -> mul -> transpose -> copy) overlaps the
                        # next burst's matmuls instead of stalling the PE
                        psb = {}
                        for jj in range(2):
                            for gu in range(2):
                                psb[(jj, gu)] = psG.tile([P, 512], f32, tag="acc", name=f"psb{jj}{gu}")
                        for k in range(KT):
                            hf, kk = divmod(k, KT // 4)
                            stat = x2T[:, k, sc * P : (sc + 1) * P]
                            for jj in range(2):
                                for gu in range(2):
                                    nc.tensor.matmul(
                                        psb[(jj, gu)][:],
                                        stat,
                                        wjs[jj][hf][:, kk, gu],
                                        start=(k == 0), stop=(k == KT - 1),
                                    )
                        for jj in range(2):
                            j = 2 * jp + jj
                            sg = sgp.tile([P, 512], bf16, tag="sg")
                            nc.scalar.activation(
                                sg[:], psb[(jj, 0)][:], AF.Silu,
                                scale=rcol2[:, sc : sc + 1],
                            )
                            ut = sgp.tile([P, 512], bf16, tag="ut")
                            nc.scalar.activation(
                                ut[:], psb[(jj, 1)][:], AF.Copy,
                                scale=rcol2[:, sc : sc + 1],
                            )
                            at = sgp.tile([P, 512], bf16, tag="at")
                            nc.vector.tensor_mul(at[:], sg[:], ut[:])
                            for c in range(4):
                                pst = psG.tile([P, P], bf16, tag="acc", name="pstr")
                                nc.tensor.transpose(
                                    pst[:], at[:, c * P : (c + 1) * P], ident[:]
                                )
                                nc.scalar.activation(
                                    a_all[:, 4 * j + c, sc * P : (sc + 1) * P],
                                    pst[:], AF.Copy,
                                )

            if "dbg_a" in t:
                nc.sync.dma_start(t["dbg_a"], a_all[:])
            with tc.tile_pool(name="psD", bufs=3, space="PSUM") as psD:
                for m in range(KT):
                    wdt = wdp.tile([P, IT, P], bf16, tag="wd")  # wd*64
                    nc.sync.dma_start(wdt[:], t["wd_t"][m])
                    psd2 = psD.tile([P, S], f32, tag="acc")
                    for i in range(IT):
                        nc.tensor.matmul(
                            psd2[:], wdt[:, i], a_all[:, i],
                            start=(i == 0), stop=(i == IT - 1),
                        )
                    ot = otp.tile([P, S], f32, tag="ot")  # = 64*out
                    nc.vector.tensor_add(ot[:], psd2[:], x2T[:, m])
                    nc.sync.dma_start(t["out_t"][:, m], ot[:])


def build_nc(depth=1):
    """Build + schedule + compile the per-core Bass program (SPMD: same program
    on all 8 cores, different input data).

    depth>1 chains the layer onto itself through internal DRAM tensors
    (timing-harness use only; the graded path uses depth=1)."""
    nc = bacc.Bacc("TRN2", target_bir_lowering=False, debug=False)
    t = {}

    def din(name, shape, dtype=bf16):
        t[name] = nc.dram_tensor(name, list(shape), dtype, kind="ExternalInput").ap()

    din("xt", (P, KT, S), bf16)
    din("x8", (P, KT, S), fp8)
    din("cosT", (P, S), f32)
    din("sinT", (P, S), f32)
    din("perm", (P, P), bf16)
    din("wq_t", (NQ // 2, P, 2, KP, 2, HD), fp8)
    din("wk_t", (NKV // 2, P, 2, KP, 2, HD), fp8)
    din("wv_t", (P, KP, 2, NKV * HD), fp8)
    din("wo_t", (KT // 2, P, 2, KP, 2, P), fp8)
    din("wgu_t", (INTER // 512, 4, P, KT // 4, 2, 512))
    din("ident", (P, P))
    din("wd_t", (KT, P, IT, P))
    t["out_t"] = nc.dram_tensor("out_t", [P, KT, S], f32, kind="ExternalOutput").ap()
    if depth == -1:  # debug build
        depth = 1
        for nm, shape, dt_ in [("dbg_h1", (P, KT, S), fp8), ("dbg_q", (P, NQ, S), bf16),
                               ("dbg_k", (P, NKV, S), bf16), ("dbg_v", (P, TCH, NKV * HD), bf16),
                               ("dbg_o8", (P, NQ, S), fp8), ("dbg_x2", (P, KT, S), bf16),
                               ("dbg_a", (P, IT, S), bf16)]:
            t[nm] = nc.dram_tensor(nm, list(shape), dt_, kind="ExternalOutput").ap()

    with tile.TileContext(nc) as tc:
        src = t["xt"]
        for d in range(depth):
            td = dict(t)
            td["xt"] = src
            if d < depth - 1:
                td["out_t"] = nc.dram_tensor(f"mid{d}", [P, KT, S], f32).ap()
                src = td["out_t"]
            _emit(tc, td, first=(d == 0))
    nc.compile()
    return nc


def _to_tiles_2d(wT, n_chunks):
    """wT: [K, N] contraction-major. -> [n_chunks, P, K//P, N//n_chunks] bf16."""
    K, N = wT.shape
    nc_cols = N // n_chunks
    r = wT.reshape(K // P, P, n_chunks, nc_cols).transpose(2, 1, 0, 3)
    return np.ascontiguousarray(r.astype(bf16_np))


def _to_pairs_fp8(wT, n_chunks):
    """wT: [K, N] contraction-major. -> [n_chunks, P, K//(2P), 2, N//n_chunks]
    fp8e4 scaled by SW, pair-interleaved for DoubleRow (half i of pair kp is
    contraction rows [(2kp+i)*P, (2kp+i+1)*P))."""
    K, N = wT.shape
    nc_cols = N // n_chunks
    r = (wT * SW).reshape(K // (2 * P), 2, P, n_chunks, nc_cols).transpose(3, 2, 0, 1, 4)
    return np.ascontiguousarray(r.astype(fp8_np))


def prep_inputs(x, pos_ids, wq, wk, wv, wo, wg, wu, wd, ln1_w, ln2_w):
    """Host-side prep: fold norm weights, transpose/tile/cast weights, gather
    rope tables, slice per-core batch. Returns list of 8 in_maps."""
    x = np.asarray(x, np.float32)
    pos_ids = np.asarray(pos_ids)
    wq = np.asarray(wq, np.float32)
    wk = np.asarray(wk, np.float32)
    wv = np.asarray(wv, np.float32)
    wo = np.asarray(wo, np.float32)
    wg = np.asarray(wg, np.float32)
    wu = np.asarray(wu, np.float32)
    wd = np.asarray(wd, np.float32)
    ln1_w = np.asarray(ln1_w, np.float32)
    ln2_w = np.asarray(ln2_w, np.float32)

    # fold RMSNorm elementwise weights into the next projections
    wqT = (wq * ln1_w[None, :]).T.copy()     # [H, NQ*HD]
    wkT = (wk * ln1_w[None, :]).T.copy()
    wvT = (wv * ln1_w[None, :]).T.copy()
    woT = wo.T.copy()                         # [NQ*HD, H]
    wgT = (wg * ln2_w[None, :]).T.copy()     # [H, INTER]
    wuT = (wu * ln2_w[None, :]).T.copy()
    wdT = (wd * SW).T.copy()                  # [INTER, H], x64 (output is 64*out)

    def _pair_heads(w):  # [n, P, KP, 2, F] -> [n//2, P, 2, KP, 2, F]
        n = w.shape[0]
        return np.ascontiguousarray(
            w.reshape(n // 2, 2, *w.shape[1:]).transpose(0, 2, 1, 3, 4, 5)
        )

    wq_t = _pair_heads(_to_pairs_fp8(wqT, NQ))   # [NQ/2, P, 2, KP, 2, HD]
    wk_t = _pair_heads(_to_pairs_fp8(wkT, NKV))
    wv_t = _to_pairs_fp8(wvT, 1)[0]              # [P, KP, 2, NKV*HD]
    wo_t = _pair_heads(_to_pairs_fp8(woT, KT))   # [KT/2, P, 2, KP, 2, P]
    # [J, hf, P, kk, {g,u}, c] = w{g,u}T[(hf*8+kk)*128+p, j*512+c]
    def _ju(w):
        r = w.reshape(4, KT // 4, P, INTER // 512, 512)
        return r.transpose(3, 0, 2, 1, 4)
    wgu_t = np.ascontiguousarray(
        np.stack([_ju(wgT), _ju(wuT)], axis=4).astype(bf16_np)
    )  # [J, 2, P, KT//2, 2, 512]
    wd_t = _to_tiles_2d(wdT, KT)             # [KT, P, IT, P]

    # rope tables
    inv_freq = 1.0 / (THETA ** (np.arange(0, HD, 2, dtype=np.float32) / HD))
    freqs = np.arange(MAX_SEQ, dtype=np.float32)[:, None] * inv_freq[None, :]
    cos = np.concatenate([np.cos(freqs), np.cos(freqs)], axis=-1)  # [MAX_SEQ, HD]
    sin = np.concatenate([np.sin(freqs), np.sin(freqs)], axis=-1)

    # swap-halves permutation (as lhsT): rot[i] = q[(i+64)%128]
    perm = np.zeros((P, P), bf16_np)
    for i in range(P):
        perm[(i + 64) % P, i] = 1.0

    shared = dict(
        perm=perm, ident=np.eye(P, dtype=bf16_np),
        wq_t=wq_t, wk_t=wk_t, wv_t=wv_t, wo_t=wo_t,
        wgu_t=wgu_t, wd_t=wd_t,
    )
    in_maps = []
    for b in range(B):
        xT = (SW * x[b]).T.reshape(KT, P, S).transpose(1, 0, 2)  # [P, KT, S] = 64*x^T
        x8 = (xT / SW).astype(fp8_np)                             # fp8(x^T), true scale
        xT = xT.astype(bf16_np)                                   # residual in bf16
        cg = (SW * cos[pos_ids[b]].T).astype(np.float32).copy()    # [HD, S] 64*cos
        sg = (SW * sin[pos_ids[b]].T).astype(np.float32).copy()   # 64*sin
        sg[: HD // 2] *= -1.0  # sign of rotate-half folded into sin
        in_maps.append(
            dict(shared, xt=np.ascontiguousarray(xT), x8=np.ascontiguousarray(x8),
                 cosT=cg, sinT=sg)
        )
    return in_maps


def unpack_output(results):
    """results: list of 8 dicts with 'out_t' [P, KT, S] = 64*out -> [B, S, H]."""
    out = np.empty((B, S, H), np.float32)
    for b in range(B):
        ot = np.asarray(results[b]["out_t"], np.float32) * (1.0 / SW)
        out[b] = ot.transpose(1, 0, 2).reshape(H, S).T
    return out


_NC_CACHE = None


def kernel(**inputs):
    global _NC_CACHE
    if _NC_CACHE is None:
        _NC_CACHE = build_nc()
    nc = _NC_CACHE
    in_maps = prep_inputs(**inputs)
    res = run_bass_kernel_spmd(nc, in_maps, core_ids=list(range(8)))
    return unpack_output(res.results)


# revision 39
# speedup vs baseline: 1.1344x; 1.1344x over previous
"""Trainium2 Bass kernel for nn_ExpertGQALayer (dense transformer layer:
RMSNorm -> GQA attention with RoPE -> residual -> RMSNorm -> SwiGLU MLP -> residual).

Sharding: pure data-parallel over batch. B=8 batch elements, 8 NeuronCores,
one batch element per core. No collectives.

Device-side dataflow keeps every activation in transposed [feature, token]
layout so that all matmul contractions sit on the SBUF partition axis.

Key speed levers vs the bf16 baseline (HW-measured: fresh-stationary bf16
MM = ~291ns, fp8 DoubleRow pair-MM = ~323ns == 1.80x per unit work):
  * q/k/v/o projections run in fp8e4 with perf_mode=DoubleRow, contracting
    two 128-chunks per instruction (weights pre-interleaved host-side,
    scaled by 64 to clear the e4m3 subnormal region).
  * All partition-axis reductions/broadcasts (rmsnorm mean, softmax
    denominator) moved off the PE onto GPSIMD partition_all_reduce, and all
    norm/softmax broadcast matmuls eliminated (the all-reduce output is
    already broadcast across partitions).
  * A global x64 scale rides the residual stream (xt64 = 64*x) so every
    fp8-weight PSUM result needs no extra descale op: the 1/64 folds into
    the RoPE cos table, the V/Q/K evacuation scales, the rsqrt chain
    (rinv/64), and the down-projection weights (wd*64); the host divides
    the output by 64.
  * norm2 square-sums accumulate eagerly inside the o-projection loop so
    the MLP starts ~immediately after the attention residual completes.

Attention scores/PV and the whole MLP stay bf16 (fp8 there fails the 2e-2
tolerance; measured on CPU: mlp-fp8 => 4.3e-2, attention-fp8 => 5e-3).
"""

import math
from contextlib import ExitStack

import ml_dtypes
import numpy as np

import concourse.mybir as mybir
import concourse.tile as tile
from concourse import bacc, bass_isa
from concourse.bass_utils import run_bass_kernel_spmd

# Problem dimensions (hardcoded per contest contract)
B, S, H = 8, 512, 2048
NQ, NKV, HD, INTER = 16, 4, 128, 8192
GROUPS = NQ // NKV
MAX_SEQ = 512
THETA = 100000.0
EPS = 1e-6
SCALE = 1.0 / math.sqrt(HD)

P = 128
KT = H // P          # 16 contraction tiles over H
KP = KT // 2         # 8 DoubleRow pair-tiles over H
IT = INTER // P      # 64 contraction tiles over INTER
TCH = S // P         # 4 token chunks

SW = 64.0            # fp8 weight scale (and the residual-stream scale)

f32 = mybir.dt.float32
bf16 = mybir.dt.bfloat16
fp8 = mybir.dt.float8e4
bf16_np = ml_dtypes.bfloat16
fp8_np = ml_dtypes.float8_e4m3

AF = mybir.ActivationFunctionType
DR = mybir.MatmulPerfMode.DoubleRow
RADD = bass_isa.ReduceOp.add


def _emit(tc, t, first=True):
    """Emit the per-core program. t: dict of DRAM APs."""
    nc = tc.nc

    with ExitStack() as octx:
        # ---- pools that live for the whole kernel ----
        glob = octx.enter_context(tc.tile_pool(name="glob", bufs=1))
        sqp = octx.enter_context(tc.tile_pool(name="sqp", bufs=3))
        accp = octx.enter_context(tc.tile_pool(name="accp", bufs=4))
        bca = octx.enter_context(tc.tile_pool(name="bca", bufs=3))
        # weight stream pool is global so phase-2 (MLP) weight prefetch can
        # begin while phase-1 pools are still live
        wst = octx.enter_context(tc.tile_pool(name="wst", bufs=4))
        psA = octx.enter_context(tc.tile_pool(name="psA", bufs=4, space="PSUM"))

        cosT = glob.tile([P, S], f32)   # 64*cos (x rinv1 = cos*r on-device)
        sinT = glob.tile([P, S], f32)    # 64*sin
        perm = glob.tile([P, P], bf16)
        x2T = glob.tile([P, KT, S], bf16)  # 64*(attention-block residual stream)
        eps_t = glob.tile([P, 1], f32)
        nc.any.memset(eps_t[:], EPS * SW * SW)  # eps for 64^2-scaled square sums

        def sq_accum(src_k, k, accs):
            """Two parallel square-sum chains (even k: ACT square + DVE add,
            odd k: DVE square + DVE add) so the per-chunk dependency chain
            never exceeds the chunk-arrival cadence. accs: [even, odd]."""
            sq = sqp.tile([P, S], f32, tag="sq")
            if k % 2 == 0:
                nc.scalar.activation(sq[:], src_k, AF.Square)
            else:
                nc.vector.tensor_mul(sq[:], src_k, src_k)
            eng = nc.vector
            if accs[k % 2] is None:
                accs[k % 2] = sq
            else:
                acc = accp.tile([P, S], f32, tag="acc")
                eng.tensor_add(acc[:], accs[k % 2][:], sq[:])
                accs[k % 2] = acc
            return accs

        def combine_accs(accs):
            acc = accp.tile([P, S], f32, tag="acc")
            nc.vector.tensor_add(acc[:], accs[0][:], accs[1][:])
            return acc

        def rinv64_from_acc(acc, sum_scale):
            """acc: [P,S] f32 partial square-sums (per-partition).
            Returns [P,S] f32 tile = rsqrt(mean_h(x^2)+eps)/64, broadcast on
            all partitions (GPSIMD all-reduce handles both the cross-partition
            sum and the broadcast). sum_scale converts the all-reduced sum to
            4096*mean (4096/H if acc holds true-scale squares, 1/H if the
            squares came from the 64x-scaled stream)."""
            ssum = bca.tile([P, S], f32, tag="ssum")
            nc.gpsimd.partition_all_reduce(ssum[:], acc[:], channels=P, reduce_op=RADD)
            srow = bca.tile([P, S], f32, tag="srow")
            # = 64*sqrt(mean+eps)
            nc.scalar.activation(srow[:], ssum[:], AF.Sqrt, bias=eps_t[:], scale=sum_scale)
            rinv = bca.tile([P, S], f32, tag="rinv")
            nc.vector.reciprocal_approx_fast(rinv[:], srow[:])
            return rinv

        # ================= phase 1: attention =================
        with ExitStack() as ctx:
            ph1 = ctx.enter_context(tc.tile_pool(name="ph1", bufs=1))
            ropep = ctx.enter_context(tc.tile_pool(name="ropep", bufs=3))
            ep = ctx.enter_context(tc.tile_pool(name="ep", bufs=7))
            esp = ctx.enter_context(tc.tile_pool(name="esp", bufs=3))

            # x8 first: the unnormalized projections consume it immediately.
            # The f32 stream (residual) is only needed by the o-projection.
            # activations/tables ride the Activation-engine DMA issue stream so
            # the SP stream is free to dispatch weight tiles immediately
            x8 = ph1.tile([P, KT, S], fp8)  # fp8(x^T), true scale
            if first:
                for k in range(0, KT, 4):
                    nc.scalar.dma_start(x8[:, k : k + 4], t["x8"][:, k : k + 4])
            nc.scalar.dma_start(cosT[:], t["cosT"])
            nc.scalar.dma_start(sinT[:], t["sinT"])
            nc.scalar.dma_start(perm[:], t["perm"])
            if first:
                xt = ph1.tile([P, KT, S], bf16)  # bf16(64*x^T); only o-proj needs it
                for k in range(0, KT, 8):
                    nc.scalar.dma_start(xt[:, k : k + 8], t["xt"][:, k : k + 8])
            else:
                # depth-chained (timing-only) layers read the f32 mid tensor and
                # derive the fp8 stream on-device
                xt = ph1.tile([P, KT, S], f32)
                for k in range(0, KT, 8):
                    nc.scalar.dma_start(xt[:, k : k + 8], t["xt"][:, k : k + 8])
                for k in range(KT):
                    if k % 2 == 0:
                        nc.scalar.activation(x8[:, k], xt[:, k], AF.Copy, scale=1.0 / SW)
                    else:
                        nc.vector.tensor_scalar_mul(x8[:, k], xt[:, k], 1.0 / SW)

            # norm1 square-sums from the (early, small) fp8 stream; the fp8
            # quantization adds ~0.1% noise to r, negligible.
            accs1 = [None, None]
            for k in range(KT):
                sq_accum(x8[:, k], k, accs1)
            rinv1 = rinv64_from_acc(combine_accs(accs1), SW * SW / H)  # rsqrt/64

            # RoPE tables with the rmsnorm scale folded in (r commutes with
            # the H-contraction): projections run on raw x8, and the per-token
            # scale r rides in via cos/sin (q/k) and the V-evacuation scale.
            cos_r = glob.tile([P, S], bf16)  # = cos * r  (cosT = 64*cos host-side)
            nc.vector.tensor_mul(cos_r[:], cosT[:], rinv1[:])
            sin_r = glob.tile([P, S], bf16)  # = sin * r   (sinT = 64*sin host-side)
            nc.vector.tensor_mul(sin_r[:], sinT[:], rinv1[:])

            q_all = ph1.tile([P, NQ, S], bf16)
            k_all = ph1.tile([P, NKV, S], bf16)
            v_all = ph1.tile([P, TCH, NKV * HD], bf16)
            o8 = ph1.tile([P, NQ, S], fp8)

            with tc.tile_pool(name="psB", bufs=2, space="PSUM") as psB:
                def project_and_rope(w_dram, n_heads, dst):
                    # head-pairs: one weight tile (4 queue-split DMAs on an
                    # alternating issue engine) feeds two heads' matmuls
                    for hp in range(n_heads // 2):
                        wt = wst.tile([P, 2, KP, 2, HD], fp8, tag="w")
                        nc.sync.dma_start(wt[:], w_dram[hp])
                        for hh in range(2):
                            psq = psA.tile([P, S], f32, tag="acc")  # = 64*q_unnorm
                            for kp in range(KP):
                                nc.tensor.matmul(
                                    psq[:], wt[:, hh, kp], x8[:, 2 * kp : 2 * kp + 2, :],
                                    start=(kp == 0), stop=(kp == KP - 1), perf_mode=DR,
                                )
                            # RoPE+norm: dst = (psq/64)*r*cos + (perm@(psq/64))*r*sin
                            qs = ropep.tile([P, S], bf16, tag="qs")
                            nc.scalar.activation(qs[:], psq[:], AF.Copy, scale=1.0 / SW)
                            psr = psB.tile([P, S], f32, tag="bc")
                            nc.tensor.matmul(psr[:], perm[:], qs[:], start=True, stop=True)
                            t1 = ropep.tile([P, S], bf16, tag="t1")
                            nc.vector.tensor_mul(t1[:], qs[:], cos_r[:])
                            t2 = ropep.tile([P, S], bf16, tag="t2")
                            nc.vector.tensor_mul(t2[:], psr[:], sin_r[:])
                            nc.vector.tensor_add(dst[:, 2 * hp + hh], t1[:], t2[:])

                project_and_rope(t["wq_t"], NQ, q_all)
                project_and_rope(t["wk_t"], NKV, k_all)

                # per-token-chunk columns of r/64 for the V evacuation scale
                one11 = nc.const_aps.tensor(1.0, (1, 1), f32)
                rcol = glob.tile([P, TCH], f32)  # rcol[:, tc] = r/64 for chunk tc
                for tc_ in range(TCH):
                    psT = psB.tile([P, 1], f32, tag="bc")
                    nc.tensor.transpose(
                        psT[:], rinv1[0:1, tc_ * P : (tc_ + 1) * P], one11
                    )
                    nc.vector.tensor_copy(rcol[:, tc_ : tc_ + 1], psT[:])

                wv_sb = ph1.tile([P, KP, 2, NKV * HD], fp8)
                nc.sync.dma_start(wv_sb[:, :4], t["wv_t"][:, :4])
                nc.sync.dma_start(wv_sb[:, 4:], t["wv_t"][:, 4:])

                for tc_ in range(TCH):
                    psv = psA.tile([P, NKV * HD], f32, tag="acc")  # 64*v_unnorm [t,d]
                    for kp in range(KP):
                        nc.tensor.matmul(
                            psv[:],
                            x8[:, 2 * kp : 2 * kp + 2, tc_ * P : (tc_ + 1) * P],
                            wv_sb[:, kp],
                            start=(kp == 0), stop=(kp == KP - 1), perf_mode=DR,
                        )
                    # v = (64*v_unnorm) * (r/64) per token-partition
                    nc.scalar.activation(
                        v_all[:, tc_], psv[:], AF.Copy, scale=rcol[:, tc_ : tc_ + 1]
                    )

            # attention per kv-group (scores get their own PSUM pool so head
            # n+1's score matmuls never wait on head n's softmax tail)
            with tc.tile_pool(name="psS", bufs=4, space="PSUM") as psS:
              for g in range(NKV):
                for h in range(g * GROUPS, (g + 1) * GROUPS):
                    e_list = []
                    for tc_ in range(TCH):
                        pss = psS.tile([P, S], f32, tag="sc")
                        nc.tensor.matmul(
                            pss[:],
                            k_all[:, g, tc_ * P : (tc_ + 1) * P],
                            q_all[:, h],
                            start=True, stop=True,
                        )
                        e = ep.tile([P, S], bf16, tag="e")
                        nc.scalar.activation(e[:], pss[:], AF.Exp, scale=SCALE)
                        e_list.append(e)
                    # PV first: keeps PE busy while DVE/GPSIMD do the denominator
                    pso = psA.tile([P, S], f32, tag="acc")
                    for tc_ in range(TCH):
                        nc.tensor.matmul(
                            pso[:],
                            v_all[:, tc_, g * HD : (g + 1) * HD],
                            e_list[tc_][:],
                            start=(tc_ == 0), stop=(tc_ == TCH - 1),
                        )
                    # denominator: DVE tree over the 4 chunks, then GPSIMD
                    # all-reduce across partitions (output already broadcast)
                    s01 = esp.tile([P, S], bf16, tag="esum")
                    nc.vector.tensor_add(s01[:], e_list[0][:], e_list[1][:])
                    s23 = esp.tile([P, S], bf16, tag="esum")
                    nc.vector.tensor_add(s23[:], e_list[2][:], e_list[3][:])
                    s03 = esp.tile([P, S], bf16, tag="esum")
                    nc.vector.tensor_add(s03[:], s01[:], s23[:])
                    den = bca.tile([P, S], f32, tag="ssum")
                    nc.gpsimd.partition_all_reduce(den[:], s03[:], channels=P, reduce_op=RADD)
                    rec = bca.tile([P, S], f32, tag="rec")
                    nc.vector.reciprocal_approx_fast(rec[:], den[:])
                    nc.vector.tensor_mul(o8[:, h], pso[:], rec[:])

            if "dbg_h1" in t:
                nc.sync.dma_start(t["dbg_h1"], x8[:])
                nc.sync.dma_start(t["dbg_q"], q_all[:])
                nc.sync.dma_start(t["dbg_k"], k_all[:])
                nc.sync.dma_start(t["dbg_v"], v_all[:])
                nc.sync.dma_start(t["dbg_o8"], o8[:])
            # o-projection + residual -> x2T (= 64*x2); eager norm2 square-sums
            accs2 = [None, None]
            for mp in range(KT // 2):
                wt = wst.tile([P, 2, KP, 2, P], fp8, tag="w")
                eng = nc.sync if mp % 2 == 0 else nc.scalar
                eng.dma_start(wt[:], t["wo_t"][mp])
                for hh in range(2):
                    m = 2 * mp + hh
                    pso = psA.tile([P, S], f32, tag="acc")  # = 64*(wo@o)
                    for jp in range(KP):
                        nc.tensor.matmul(
                            pso[:], wt[:, hh, jp], o8[:, 2 * jp : 2 * jp + 2, :],
                            start=(jp == 0), stop=(jp == KP - 1), perf_mode=DR,
                        )
                    nc.vector.tensor_add(x2T[:, m], pso[:], xt[:, m])
                    sq_accum(x2T[:, m], m, accs2)

        if "dbg_x2" in t:
            nc.sync.dma_start(t["dbg_x2"], x2T[:])
        # ================= phase 2: MLP =================
        with ExitStack() as ctx:
            ph2 = ctx.enter_context(tc.tile_pool(name="ph2", bufs=1))
            wgup = ctx.enter_context(tc.tile_pool(name="wgup", bufs=2))
            wdp = ctx.enter_context(tc.tile_pool(name="wdp", bufs=2))
            sgp = ctx.enter_context(tc.tile_pool(name="sgp", bufs=2))
            otp = ctx.enter_context(tc.tile_pool(name="otp", bufs=2))

            # rinv2 = r2/64; the gate/up matmuls run on the 64x-scaled x2T
            # directly (r2 commutes with the H-contraction), so multiplying
            # the PSUM result by rinv2 lands exactly on true scale.
            rinv2 = rinv64_from_acc(combine_accs(accs2), 1.0 / H)
            a_all = ph2.tile([P, IT, S], bf16)
            for i in range(IT):
                wgu = wgup.tile([P, 2, KT, P], bf16, tag="w2")
                nc.sync.dma_start(wgu[:, 0], t["wgu_t"][i, :, 0])
                nc.sync.dma_start(wgu[:, 1], t["wgu_t"][i, :, 1])
                wgt = wgu[:, 0]
                wut = wgu[:, 1]
                psg = psA.tile([P, S], f32, tag="acc")
                psu = psA.tile([P, S], f32, tag="acc")
                for k in range(KT):
                    nc.tensor.matmul(
                        psg[:], wgt[:, k], x2T[:, k], start=(k == 0), stop=(k == KT - 1)
                    )
                for k in range(KT):
                    nc.tensor.matmul(
                        psu[:], wut[:, k], x2T[:, k], start=(k == 0), stop=(k == KT - 1)
                    )
                gt = sgp.tile([P, S], f32, tag="gt")
                nc.vector.tensor_mul(gt[:], psg[:], rinv2[:])
                ut = sgp.tile([P, S], bf16, tag="ut")
                nc.vector.tensor_mul(ut[:], psu[:], rinv2[:])
                sg = sgp.tile([P, S], bf16, tag="sg")
                nc.scalar.activation(sg[:], gt[:], AF.Silu)
                nc.vector.tensor_mul(a_all[:, i], ut[:], sg[:])

            for m in range(KT):
                wdt = wdp.tile([P, IT, P], bf16, tag="wd")  # wd*64
                nc.sync.dma_start(wdt[:], t["wd_t"][m])
                psd2 = psA.tile([P, S], f32, tag="acc")
                for i in range(IT):
                    nc.tensor.matmul(
                        psd2[:], wdt[:, i], a_all[:, i], start=(i == 0), stop=(i == IT - 1)
                    )
                ot = otp.tile([P, S], f32, tag="ot")  # = 64*out
                nc.vector.tensor_add(ot[:], psd2[:], x2T[:, m])
                nc.sync.dma_start(t["out_t"][:, m], ot[:])


def build_nc(depth=1):
    """Build + schedule + compile the per-core Bass program (SPMD: same program
    on all 8 cores, different input data).

    depth>1 chains the layer onto itself through internal DRAM tensors
    (timing-harness use only; the graded path uses depth=1)."""
    nc = bacc.Bacc("TRN2", target_bir_lowering=False, debug=False)
    t = {}

    def din(name, shape, dtype=bf16):
        t[name] = nc.dram_tensor(name, list(shape), dtype, kind="ExternalInput").ap()

    din("xt", (P, KT, S), bf16)
    din("x8", (P, KT, S), fp8)
    din("cosT", (P, S), f32)
    din("sinT", (P, S), f32)
    din("perm", (P, P), bf16)
    din("wq_t", (NQ // 2, P, 2, KP, 2, HD), fp8)
    din("wk_t", (NKV // 2, P, 2, KP, 2, HD), fp8)
    din("wv_t", (P, KP, 2, NKV * HD), fp8)
    din("wo_t", (KT // 2, P, 2, KP, 2, P), fp8)
    din("wgu_t", (IT, P, 2, KT, P))
    din("wd_t", (KT, P, IT, P))
    t["out_t"] = nc.dram_tensor("out_t", [P, KT, S], f32, kind="ExternalOutput").ap()
    if depth == -1:  # debug build
        depth = 1
        for nm, shape, dt_ in [("dbg_h1", (P, KT, S), fp8), ("dbg_q", (P, NQ, S), bf16),
                               ("dbg_k", (P, NKV, S), bf16), ("dbg_v", (P, TCH, NKV * HD), bf16),
                               ("dbg_o8", (P, NQ, S), fp8), ("dbg_x2", (P, KT, S), bf16)]:
            t[nm] = nc.dram_tensor(nm, list(shape), dt_, kind="ExternalOutput").ap()

    with tile.TileContext(nc) as tc:
        src = t["xt"]
        for d in range(depth):
            td = dict(t)
            td["xt"] = src
            if d < depth - 1:
                td["out_t"] = nc.dram_tensor(f"mid{d}", [P, KT, S], f32).ap()
                src = td["out_t"]
            _emit(tc, td, first=(d == 0))
    nc.compile()
    return nc


def _to_tiles_2d(wT, n_chunks):
    """wT: [K, N] contraction-major. -> [n_chunks, P, K//P, N//n_chunks] bf16."""
    K, N = wT.shape
    nc_cols = N // n_chunks
    r = wT.reshape(K // P, P, n_chunks, nc_cols).transpose(2, 1, 0, 3)
    return np.ascontiguousarray(r.astype(bf16_np))


def _to_pairs_fp8(wT, n_chunks):
    """wT: [K, N] contraction-major. -> [n_chunks, P, K//(2P), 2, N//n_chunks]
    fp8e4 scaled by SW, pair-interleaved for DoubleRow (half i of pair kp is
    contraction rows [(2kp+i)*P, (2kp+i+1)*P))."""
    K, N = wT.shape
    nc_cols = N // n_chunks
    r = (wT * SW).reshape(K // (2 * P), 2, P, n_chunks, nc_cols).transpose(3, 2, 0, 1, 4)
    return np.ascontiguousarray(r.astype(fp8_np))


def prep_inputs(x, pos_ids, wq, wk, wv, wo, wg, wu, wd, ln1_w, ln2_w):
    """Host-side prep: fold norm weights, transpose/tile/cast weights, gather
    rope tables, slice per-core batch. Returns list of 8 in_maps."""
    x = np.asarray(x, np.float32)
    pos_ids = np.asarray(pos_ids)
    wq = np.asarray(wq, np.float32)
    wk = np.asarray(wk, np.float32)
    wv = np.asarray(wv, np.float32)
    wo = np.asarray(wo, np.float32)
    wg = np.asarray(wg, np.float32)
    wu = np.asarray(wu, np.float32)
    wd = np.asarray(wd, np.float32)
    ln1_w = np.asarray(ln1_w, np.float32)
    ln2_w = np.asarray(ln2_w, np.float32)

    # fold RMSNorm elementwise weights into the next projections
    wqT = (wq * ln1_w[None, :]).T.copy()     # [H, NQ*HD]
    wkT = (wk * ln1_w[None, :]).T.copy()
    wvT = (wv * ln1_w[None, :]).T.copy()
    woT = wo.T.copy()                         # [NQ*HD, H]
    wgT = (wg * ln2_w[None, :]).T.copy()     # [H, INTER]
    wuT = (wu * ln2_w[None, :]).T.copy()
    wdT = (wd * SW).T.copy()                  # [INTER, H], x64 (output is 64*out)

    def _pair_heads(w):  # [n, P, KP, 2, F] -> [n//2, P, 2, KP, 2, F]
        n = w.shape[0]
        return np.ascontiguousarray(
            w.reshape(n // 2, 2, *w.shape[1:]).transpose(0, 2, 1, 3, 4, 5)
        )

    wq_t = _pair_heads(_to_pairs_fp8(wqT, NQ))   # [NQ/2, P, 2, KP, 2, HD]
    wk_t = _pair_heads(_to_pairs_fp8(wkT, NKV))
    wv_t = _to_pairs_fp8(wvT, 1)[0]              # [P, KP, 2, NKV*HD]
    wo_t = _pair_heads(_to_pairs_fp8(woT, KT))   # [KT/2, P, 2, KP, 2, P]
    wg_t = _to_tiles_2d(wgT, IT)
    wu_t = _to_tiles_2d(wuT, IT)
    wgu_t = np.ascontiguousarray(np.stack([wg_t, wu_t], axis=2))  # [IT, P, 2, KT, P]
    wd_t = _to_tiles_2d(wdT, KT)             # [KT, P, IT, P]

    # rope tables
    inv_freq = 1.0 / (THETA ** (np.arange(0, HD, 2, dtype=np.float32) / HD))
    freqs = np.arange(MAX_SEQ, dtype=np.float32)[:, None] * inv_freq[None, :]
    cos = np.concatenate([np.cos(freqs), np.cos(freqs)], axis=-1)  # [MAX_SEQ, HD]
    sin = np.concatenate([np.sin(freqs), np.sin(freqs)], axis=-1)

    # swap-halves permutation (as lhsT): rot[i] = q[(i+64)%128]
    perm = np.zeros((P, P), bf16_np)
    for i in range(P):
        perm[(i + 64) % P, i] = 1.0

    shared = dict(
        perm=perm,
        wq_t=wq_t, wk_t=wk_t, wv_t=wv_t, wo_t=wo_t,
        wgu_t=wgu_t, wd_t=wd_t,
    )
    in_maps = []
    for b in range(B):
        xT = (SW * x[b]).T.reshape(KT, P, S).transpose(1, 0, 2)  # [P, KT, S] = 64*x^T
        x8 = (xT / SW).astype(fp8_np)                             # fp8(x^T), true scale
        xT = xT.astype(bf16_np)                                   # residual in bf16
        cg = (SW * cos[pos_ids[b]].T).astype(np.float32).copy()    # [HD, S] 64*cos
        sg = (SW * sin[pos_ids[b]].T).astype(np.float32).copy()   # 64*sin
        sg[: HD // 2] *= -1.0  # sign of rotate-half folded into sin
        in_maps.append(
            dict(shared, xt=np.ascontiguousarray(xT), x8=np.ascontiguousarray(x8),
                 cosT=cg, sinT=sg)
        )
    return in_maps


def unpack_output(results):
    """results: list of 8 dicts with 'out_t' [P, KT, S] = 64*out -> [B, S, H]."""
    out = np.empty((B, S, H), np.float32)
    for b in range(B):
        ot = np.asarray(results[b]["out_t"], np.float32) * (1.0 / SW)
        out[b] = ot.transpose(1, 0, 2).reshape(H, S).T
    return out


_NC_CACHE = None


def kernel(**inputs):
    global _NC_CACHE
    if _NC_CACHE is None:
        _NC_CACHE = build_nc()
    nc = _NC_CACHE
    in_maps = prep_inputs(**inputs)
    res = run_bass_kernel_spmd(nc, in_maps, core_ids=list(range(8)))
    return unpack_output(res.results)
